# revision 1
# baseline (speedup 1.0000x reference)
"""Two-layer GCN (PyG GCNConv x2 + ReLU) on 8 Trainium2 NeuronCores.

Sharding: nodes are range-partitioned across the 8 cores (6250 each).
Each core computes h = dinv * (x_shard @ W) for its nodes, the per-node
feature tables are AllGathered, and each core then processes the edges
whose destination falls in its node range: a batched indirect row gather
of source features (dma_gather) followed by a one-hot-matmul scatter-add
into PSUM over destination tiles (edges pre-sorted by destination on the
host). Weights are replicated.
"""

import hashlib
import sys

import ml_dtypes
import numpy as np

sys.path.insert(0, "/opt/trn_rl_repo")

import concourse.bacc as bacc
import concourse.mybir as mybir
import concourse.tile as tile
from concourse.bass_utils import run_bass_kernel_spmd

N = 50000
F0 = 768
FM = 256
N_CORES = 8
NPC = N // N_CORES  # 6250
TILES = (NPC + 127) // 128  # 49
SPLIT = 25000  # int16-safe gather table split
SGT = 2  # dst tiles per gather super-group

F32 = mybir.dt.float32
BF16 = mybir.dt.bfloat16
I32 = mybir.dt.int32
I16 = mybir.dt.int16

_cache = {}


def _make_plan(edge_index):
    src = np.asarray(edge_index[0], dtype=np.int64)
    dst = np.asarray(edge_index[1], dtype=np.int64)
    deg = (np.bincount(dst, minlength=N) + 1).astype(np.float64)
    dinv = (1.0 / np.sqrt(deg)).astype(np.float32)

    loops = np.arange(N, dtype=np.int64)
    s_all = np.concatenate([src, loops])
    d_all = np.concatenate([dst, loops])

    core = d_all // NPC
    dloc = d_all - core * NPC
    t_all = dloc // 128
    p_all = dloc - t_all * 128
    h_all = (s_all >= SPLIT).astype(np.int64)

    # group id within a core: g = t*2 + h ; groups ordered per super-group:
    # sg -> [lo chunks of its tiles (t asc)], then [hi chunks (t asc)]
    n_sgs = (TILES + SGT - 1) // SGT
    group_seq = []  # (t, h) in slot-array order
    for sg in range(n_sgs):
        ts = range(sg * SGT, min((sg + 1) * SGT, TILES))
        for t in ts:
            group_seq.append((t, 0))
        for t in ts:
            group_seq.append((t, 1))

    # per-core counts per (t, h)
    counts = np.zeros((N_CORES, TILES, 2), np.int64)
    flatg = (core * TILES * 2 + t_all * 2 + h_all).astype(np.int64)
    bc = np.bincount(flatg, minlength=N_CORES * TILES * 2)
    counts = bc.reshape(N_CORES, TILES, 2)
    nch = np.maximum(1, (counts.max(axis=0) + 127) // 128)  # [TILES, 2] chunks

    # chunk/slot base per group in slot-array order
    gbase_chunk = {}
    acc = 0
    for (t, h) in group_seq:
        gbase_chunk[(t, h)] = acc
        acc += int(nch[t, h])
    totc = acc
    tot = totc * 128

    # super-group metadata (shared across cores)
    sgs = []
    for sg in range(n_sgs):
        ts = list(range(sg * SGT, min((sg + 1) * SGT, TILES)))
        lo0 = gbase_chunk[(ts[0], 0)]
        nlo = sum(int(nch[t, 0]) for t in ts)
        hi0 = gbase_chunk[(ts[0], 1)]
        nhi = sum(int(nch[t, 1]) for t in ts)
        tl = []
        for t in ts:
            chunks = []
            for k in range(int(nch[t, 0])):
                gc = gbase_chunk[(t, 0)] + k
                chunks.append((gc, 0, gc - lo0))
            for k in range(int(nch[t, 1])):
                gc = gbase_chunk[(t, 1)] + k
                chunks.append((gc, 1, gc - hi0))
            tl.append((t, chunks))
        sgs.append({"lo0": lo0, "nlo": nlo, "hi0": hi0, "nhi": nhi, "tiles": tl})

    # per-core slot arrays
    idx_arrs, dst_arrs, scale_arrs = [], [], []
    order_key = t_all * 2 + h_all
    for c in range(N_CORES):
        sel = np.nonzero(core == c)[0]
        k = order_key[sel]
        o = np.argsort(k, kind="stable")
        sel = sel[o]
        k = k[o]
        # rank within group
        grp_counts = np.bincount(k, minlength=TILES * 2)
        grp_start = np.concatenate([[0], np.cumsum(grp_counts)[:-1]])
        rank = np.arange(len(sel)) - grp_start[k]
        tt = t_all[sel]
        hh = h_all[sel]
        slot = (
            np.array([gbase_chunk[(int(t), int(h))] for t, h in zip(tt, hh)])
            * 128
            + rank
        )
        idx_flat = np.zeros(tot, np.int16)
        dst_flat = np.full(tot, -1.0, np.float32)
        sc_flat = np.zeros(tot, np.float32)
        idx_flat[slot] = (s_all[sel] - hh * SPLIT).astype(np.int16)
        dst_flat[slot] = p_all[sel].astype(np.float32)
        sc_flat[slot] = dinv[d_all[sel]]

        idx16 = np.zeros((32, tot // 16), np.int16)
        idx16[16:32, :] = idx_flat.reshape(tot // 16, 16).T
        idx_arrs.append(idx16)
        # host-built one-hot scatter tiles: st[gc, r, c] = dinv[d] where
        # slot gc*128+r has destination partition c (0 elsewhere)
        st = np.zeros((totc, 128, 128), np.float32)
        valid = dst_flat >= 0
        slots = np.nonzero(valid)[0]
        st[slots // 128, slots % 128, dst_flat[valid].astype(np.int64)] = sc_flat[
            valid
        ]
        dst_arrs.append(
            np.ascontiguousarray(st.transpose(1, 0, 2).reshape(128, totc * 128)).astype(
                ml_dtypes.bfloat16
            )
        )
        scale_arrs.append(None)

    # per-core dinv table [128, TILES]
    dinv_loc = []
    for c in range(N_CORES):
        dl = np.zeros((128, TILES), np.float32)
        v = dinv[c * NPC : (c + 1) * NPC]
        for t in range(TILES):
            seg = v[t * 128 : (t + 1) * 128]
            dl[: len(seg), t] = seg
        dinv_loc.append(dl)

    return {
        "sgs": sgs,
        "totc": totc,
        "tot": tot,
        "idx": idx_arrs,
        "dst": dst_arrs,
        "scale": scale_arrs,
        "dinv_loc": dinv_loc,
    }


def _build(plan, phases=(1, 2, 3)):
    totc = plan["totc"]
    tot = plan["tot"]
    idxc = tot // 16

    nc = bacc.Bacc(
        "TRN2", target_bir_lowering=False, debug=False, num_devices=N_CORES
    )
    xT = nc.dram_tensor("xT", [F0, NPC], BF16, kind="ExternalInput")
    w1 = nc.dram_tensor("w1", [F0, FM], BF16, kind="ExternalInput")
    w2 = nc.dram_tensor("w2", [FM, FM], BF16, kind="ExternalInput")
    b1c = nc.dram_tensor("b1c", [128, 2], F32, kind="ExternalInput")
    b2bc = nc.dram_tensor("b2bc", [128, FM], F32, kind="ExternalInput")
    dinvl = nc.dram_tensor("dinvl", [128, TILES], F32, kind="ExternalInput")
    idxs = nc.dram_tensor("idxs", [32, idxc], I16, kind="ExternalInput")
    stl = nc.dram_tensor("stl", [128, totc * 128], BF16, kind="ExternalInput")
    out = nc.dram_tensor("out", [NPC, FM], F32, kind="ExternalOutput")

    K0 = F0 // 128  # 6

    with tile.TileContext(nc) as tc:
        with (
            tc.tile_pool(name="const", bufs=1) as cpool,
            tc.tile_pool(name="sbuf", bufs=3) as sbuf,
            tc.tile_pool(name="gbuf", bufs=2) as gbuf,
            tc.tile_pool(name="stbuf", bufs=2) as stbuf,
            tc.tile_pool(name="psum", bufs=2, space="PSUM") as psum,
            tc.tile_pool(name="dram", bufs=1, space="DRAM") as dram,
        ):
            # ---- persistent tiles ----
            w1t = cpool.tile([128, K0, FM], BF16)
            nc.sync.dma_start(
                out=w1t[:], in_=w1[:].rearrange("(k p) f -> p k f", p=128)
            )
            w2t = cpool.tile([128, 2, FM], BF16)
            nc.sync.dma_start(
                out=w2t[:], in_=w2[:].rearrange("(k p) f -> p k f", p=128)
            )
            b1t = cpool.tile([128, 2], F32)
            nc.sync.dma_start(out=b1t[:], in_=b1c[:])
            b2t = cpool.tile([128, FM], F32)
            nc.sync.dma_start(out=b2t[:], in_=b2bc[:])
            dvt = cpool.tile([128, TILES], F32)
            nc.sync.dma_start(out=dvt[:], in_=dinvl[:])
            idx_t = cpool.tile([32, idxc], I16)
            nc.sync.dma_start(out=idx_t[:], in_=idxs[:])
            xfull = cpool.tile([128, K0, NPC], BF16)
            nc.sync.dma_start(
                out=xfull[:], in_=xT[:].rearrange("(k p) n -> p k n", p=128)
            )

            h1shard = dram.tile([NPC, FM], BF16)
            h1full = dram.tile([N, FM], BF16, addr_space="Shared")
            h2shard = dram.tile([NPC, FM], BF16)
            h2full = dram.tile([N, FM], BF16, addr_space="Shared")

            def tw_of(t):
                return min(128, NPC - t * 128)

            # ---- P1: h1 = dinv * (x @ W1) ----
            for t in range(TILES if 1 in phases else 0):
                tw = tw_of(t)
                ps = psum.tile([128, FM], F32, tag="mmps", space="PSUM")
                for k in range(K0):
                    nc.tensor.matmul(
                        out=ps[:tw, :],
                        lhsT=xfull[:, k, t * 128 : t * 128 + tw],
                        rhs=w1t[:, k, :],
                        start=(k == 0),
                        stop=(k == K0 - 1),
                    )
                hs = sbuf.tile([128, FM], BF16, tag="hs")
                nc.scalar.activation(
                    out=hs[:tw, :],
                    in_=ps[:tw, :],
                    func=mybir.ActivationFunctionType.Copy,
                    scale=dvt[:tw, t : t + 1],
                )
                nc.sync.dma_start(
                    out=h1shard[t * 128 : t * 128 + tw, :], in_=hs[:tw, :]
                )

            if 1 in phases and 2 in phases:
                nc.gpsimd.collective_compute(
                    "AllGather",
                    mybir.AluOpType.bypass,
                    replica_groups=[list(range(N_CORES))],
                    ins=[h1shard.opt()],
                    outs=[h1full.opt()],
                )

            # ---- P2: layer-1 message passing + layer-2 dense ----
            def gather_sg(sg, table):
                glo = ghi = None
                if sg["nlo"]:
                    glo = gbuf.tile([128, sg["nlo"], FM], BF16, tag="glo")
                    nc.gpsimd.dma_gather(
                        glo[:],
                        table[0:SPLIT, :],
                        idx_t[:, 8 * sg["lo0"] : 8 * (sg["lo0"] + sg["nlo"])],
                        sg["nlo"] * 128,
                        sg["nlo"] * 128,
                        FM,
                        single_packet=False,
                    )
                if sg["nhi"]:
                    ghi = gbuf.tile([128, sg["nhi"], FM], BF16, tag="ghi")
                    nc.gpsimd.dma_gather(
                        ghi[:],
                        table[SPLIT:N, :],
                        idx_t[:, 8 * sg["hi0"] : 8 * (sg["hi0"] + sg["nhi"])],
                        sg["nhi"] * 128,
                        sg["nhi"] * 128,
                        FM,
                        single_packet=False,
                    )
                return glo, ghi

            def load_st(sg):
                ntot = sg["nlo"] + sg["nhi"]
                st = stbuf.tile([128, ntot * 128], BF16, tag="st")
                nc.sync.dma_start(
                    out=st[:],
                    in_=stl[:, sg["lo0"] * 128 : (sg["lo0"] + ntot) * 128],
                )
                return st

            for sg in plan["sgs"] if 2 in phases else []:
                glo, ghi = gather_sg(sg, h1full)
                st = load_st(sg)
                for t, chunks in sg["tiles"]:
                    tw = tw_of(t)
                    ps0 = psum.tile([128, 128], F32, tag="psT0", space="PSUM")
                    ps1 = psum.tile([128, 128], F32, tag="psT1", space="PSUM")
                    nchunks = len(chunks)
                    for i, (gc, buf, col) in enumerate(chunks):
                        g = glo if buf == 0 else ghi
                        off = (gc - sg["lo0"]) * 128
                        nc.tensor.matmul(
                            out=ps0[:, :tw],
                            lhsT=g[:, col, 0:128],
                            rhs=st[:, off : off + tw],
                            start=(i == 0),
                            stop=(i == nchunks - 1),
                        )
                        nc.tensor.matmul(
                            out=ps1[:, :tw],
                            lhsT=g[:, col, 128:256],
                            rhs=st[:, off : off + tw],
                            start=(i == 0),
                            stop=(i == nchunks - 1),
                        )
                    x1a = sbuf.tile([128, 128], BF16, tag="x1a")
                    x1b = sbuf.tile([128, 128], BF16, tag="x1b")
                    nc.scalar.activation(
                        out=x1a[:, :tw],
                        in_=ps0[:, :tw],
                        func=mybir.ActivationFunctionType.Relu,
                        bias=b1t[:, 0:1],
                    )
                    nc.scalar.activation(
                        out=x1b[:, :tw],
                        in_=ps1[:, :tw],
                        func=mybir.ActivationFunctionType.Relu,
                        bias=b1t[:, 1:2],
                    )
                    ps2 = psum.tile([128, FM], F32, tag="mmps", space="PSUM")
                    nc.tensor.matmul(
                        out=ps2[:tw, :],
                        lhsT=x1a[:, :tw],
                        rhs=w2t[:, 0, :],
                        start=True,
                        stop=False,
                    )
                    nc.tensor.matmul(
                        out=ps2[:tw, :],
                        lhsT=x1b[:, :tw],
                        rhs=w2t[:, 1, :],
                        start=False,
                        stop=True,
                    )
                    h2s = sbuf.tile([128, FM], BF16, tag="hs")
                    nc.scalar.activation(
                        out=h2s[:tw, :],
                        in_=ps2[:tw, :],
                        func=mybir.ActivationFunctionType.Copy,
                        scale=dvt[:tw, t : t + 1],
                    )
                    nc.sync.dma_start(
                        out=h2shard[t * 128 : t * 128 + tw, :], in_=h2s[:tw, :]
                    )

            if 2 in phases and 3 in phases:
                nc.gpsimd.collective_compute(
                    "AllGather",
                    mybir.AluOpType.bypass,
                    replica_groups=[list(range(N_CORES))],
                    ins=[h2shard.opt()],
                    outs=[h2full.opt()],
                )

            # ---- P3: layer-2 message passing + bias ----
            for sg in plan["sgs"] if 3 in phases else []:
                glo, ghi = gather_sg(sg, h2full)
                st = load_st(sg)
                for t, chunks in sg["tiles"]:
                    tw = tw_of(t)
                    ps = psum.tile([128, FM], F32, tag="mmps", space="PSUM")
                    nchunks = len(chunks)
                    for i, (gc, buf, col) in enumerate(chunks):
                        g = glo if buf == 0 else ghi
                        off = (gc - sg["lo0"]) * 128
                        nc.tensor.matmul(
                            out=ps[:tw, :],
                            lhsT=st[:, off : off + tw],
                            rhs=g[:, col, :],
                            start=(i == 0),
                            stop=(i == nchunks - 1),
                        )
                    ot = sbuf.tile([128, FM], F32, tag="ot")
                    nc.vector.tensor_add(
                        out=ot[:tw, :], in0=ps[:tw, :], in1=b2t[:tw, :]
                    )
                    nc.sync.dma_start(
                        out=out[t * 128 : t * 128 + tw, :], in_=ot[:tw, :]
                    )
    nc.compile()
    return nc


def _prep(plan, x, W1, b1, W2, b2):
    x = np.asarray(x, np.float32)
    W1 = np.asarray(W1, np.float32).astype(ml_dtypes.bfloat16)
    W2 = np.asarray(W2, np.float32).astype(ml_dtypes.bfloat16)
    b1 = np.asarray(b1, np.float32)
    b2 = np.asarray(b2, np.float32)
    b1c = np.ascontiguousarray(b1.reshape(2, 128).T)
    b2bc = np.ascontiguousarray(np.broadcast_to(b2[None, :], (128, FM)))
    in_maps = []
    for c in range(N_CORES):
        xs = x[c * NPC : (c + 1) * NPC]
        in_maps.append(
            {
                "xT": np.ascontiguousarray(xs.T).astype(ml_dtypes.bfloat16),
                "w1": W1,
                "w2": W2,
                "b1c": b1c,
                "b2bc": b2bc,
                "dinvl": plan["dinv_loc"][c],
                "idxs": plan["idx"][c],
                "stl": plan["dst"][c],
            }
        )
    return in_maps


def kernel(x, edge_index, W1, b1, W2, b2):
    key = hashlib.sha256(np.asarray(edge_index).tobytes()).hexdigest()
    if key not in _cache:
        plan = _make_plan(edge_index)
        nc = _build(plan)
        _cache[key] = (plan, nc)
    plan, nc = _cache[key]
    in_maps = _prep(plan, x, W1, b1, W2, b2)

    last_err = None
    for _ in range(3):
        try:
            res = run_bass_kernel_spmd(
                nc, in_maps, core_ids=list(range(N_CORES))
            )
            break
        except Exception as e:  # transient NRT failures
            last_err = e
    else:
        raise last_err
    return np.concatenate([res.results[c]["out"] for c in range(N_CORES)], axis=0)



# revision 3
# speedup vs baseline: 1.2207x; 1.2207x over previous
"""Two-layer GCN (PyG GCNConv x2 + ReLU) on 8 Trainium2 NeuronCores.

v2. Nodes range-partitioned across 8 cores (6250 each). Per layer:
h = x @ W computed per-shard, AllGathered in two halves (A = first 3125
rows of each shard, B = rest) so gathers against half A overlap the
collective for half B. Message passing: per destination-super-group
(SGT tiles), batched indirect row gathers (dma_gather, edge-exact slot
packing with trailing zero-idx padding) + one-hot-matmul scatter-add
into per-tile PSUM. Edge norm dinv[s]*dinv[d] is folded into the
host-built one-hot tables; self-loop terms enter PSUM via an
identity-matmul of the locally-kept h rows (layer 1) / DVE adds
(layer 2), so self-loops are never gathered.
"""

import hashlib
import sys

import ml_dtypes
import numpy as np

sys.path.insert(0, "/opt/trn_rl_repo")

import concourse.bacc as bacc
import concourse.mybir as mybir
import concourse.tile as tile
from concourse.bass_utils import run_bass_kernel_spmd

N = 50000
F0 = 768
FM = 256
N_CORES = 8
NPC = N // N_CORES  # 6250
HALF = NPC // 2  # 3125 -> tables of 25000 rows (int16-safe)
TILES = (NPC + 127) // 128  # 49
SGT = 4
NSG = (TILES + SGT - 1) // SGT  # 13
NT = N_CORES * HALF  # 25000 rows per gather table

F32 = mybir.dt.float32
BF16 = mybir.dt.bfloat16
I32 = mybir.dt.int32
I16 = mybir.dt.int16

_cache = {}


def _make_plan(edge_index):
    src = np.asarray(edge_index[0], dtype=np.int64)
    dst = np.asarray(edge_index[1], dtype=np.int64)
    E = src.shape[0]
    deg = (np.bincount(dst, minlength=N) + 1).astype(np.float64)
    dinv = (1.0 / np.sqrt(deg)).astype(np.float64)
    norm = (dinv[src] * dinv[dst]).astype(np.float32)

    core = dst // NPC
    dloc = dst - core * NPC
    t_all = dloc // 128  # tile within core
    p_all = dloc - t_all * 128  # partition within tile
    sg_all = t_all // SGT
    sl = src % NPC
    half = (sl >= HALF).astype(np.int64)
    tidx = (src // NPC) * HALF + (sl - half * HALF)  # row in table A/B

    calls = []  # one per (sg, half): static metadata shared across cores
    # per-core containers
    idx_lists = [[] for _ in range(N_CORES)]
    st_cols_data = [[] for _ in range(N_CORES)]  # list of (col_off, r, p, v) arrs

    # group edges per (core, sg, half), sorted by tile
    key = ((core * NSG + sg_all) * 2 + half) * TILES + t_all
    order = np.argsort(key, kind="stable")
    so_src = tidx[order]
    so_key = key[order]
    so_t = t_all[order]
    so_p = p_all[order]
    so_norm = norm[order]
    # boundaries per (core, sg, half)
    gkey = (core * NSG + sg_all) * 2 + half
    gk_sorted = gkey[order]

    st_total_cols = 0
    idx_total_cols = 0
    for sg in range(NSG):
        tiles = list(range(sg * SGT, min((sg + 1) * SGT, TILES)))
        for h in (0, 1):
            # per-core edge ranges for this call
            pc = []
            for c in range(N_CORES):
                g = (c * NSG + sg) * 2 + h
                lo = np.searchsorted(gk_sorted, g, side="left")
                hi = np.searchsorted(gk_sorted, g, side="right")
                pc.append((lo, hi))
            cnt_max = max(hi - lo for lo, hi in pc)
            nch = max(1, (cnt_max + 127) // 128)
            rows = nch * 128
            # per (tile): union chunk range over cores
            tile_meta = []
            col_off = st_total_cols
            blocks = {}  # (t) -> (kmin, kmax)
            for t in tiles:
                kmin, kmax = None, None
                for c in range(N_CORES):
                    lo, hi = pc[c]
                    tl = np.searchsorted(so_t[lo:hi], t, side="left")
                    th = np.searchsorted(so_t[lo:hi], t, side="right")
                    if th > tl:
                        k0, k1 = tl // 128, (th - 1) // 128
                        kmin = k0 if kmin is None else min(kmin, k0)
                        kmax = k1 if kmax is None else max(kmax, k1)
                if kmin is None:
                    kmin, kmax = 0, 0  # degenerate; still emit one block
                blocks[t] = (kmin, kmax)
            for t in tiles:
                kmin, kmax = blocks[t]
                ks = list(range(kmin, kmax + 1))
                tile_meta.append((t, ks, col_off))
                col_off += len(ks)
            call = {
                "sg": sg,
                "half": h,
                "nch": nch,
                "rows": rows,
                "idx_off": idx_total_cols,  # in int16 columns (rows/16)
                "st_off": st_total_cols,  # in 128-col blocks
                "tiles": tile_meta,
                "nblocks": col_off - st_total_cols,
            }
            calls.append(call)
            # per-core idx + st data
            for c in range(N_CORES):
                lo, hi = pc[c]
                cnt = hi - lo
                idx = np.zeros(rows, np.int16)
                idx[:cnt] = so_src[lo:hi].astype(np.int16)
                idx_lists[c].append(idx)
                # st entries: slot s (< cnt): tile so_t, chunk k=s//128, r=s%128
                s_arr = np.arange(cnt)
                kk = s_arr // 128
                rr = s_arr - kk * 128
                tt = so_t[lo:hi]
                pp = so_p[lo:hi]
                vv = so_norm[lo:hi]
                # block column for (t, k): col_off(t) + (k - kmin(t))
                bcol = np.empty(cnt, np.int64)
                for t, ks, coff in tile_meta:
                    m = tt == t
                    bcol[m] = coff + (kk[m] - ks[0])
                st_cols_data[c].append((bcol, rr, pp, vv))
            st_total_cols = col_off
            idx_total_cols += rows // 16

    # build per-core packed arrays
    idx_arrs = []
    st_arrs = []
    for c in range(N_CORES):
        flat = np.concatenate(idx_lists[c])
        i16 = np.zeros((32, idx_total_cols), np.int16)
        i16[16:32, :] = flat.reshape(idx_total_cols, 16).T
        idx_arrs.append(i16)
        st = np.zeros((st_total_cols, 128, 128), np.float32)
        for (bcol, rr, pp, vv) in st_cols_data[c]:
            st[bcol, rr, pp] = vv
        st_arrs.append(
            np.ascontiguousarray(
                st.transpose(1, 0, 2).reshape(128, st_total_cols * 128)
            ).astype(ml_dtypes.bfloat16)
        )

    # per-core dinv^2 table [128, TILES] (0 beyond tile width)
    dinvsq = []
    d2 = (dinv * dinv).astype(np.float32)
    for c in range(N_CORES):
        dl = np.zeros((128, TILES), np.float32)
        v = d2[c * NPC : (c + 1) * NPC]
        for t in range(TILES):
            seg = v[t * 128 : (t + 1) * 128]
            dl[: len(seg), t] = seg
        dinvsq.append(dl)

    return {
        "calls": calls,
        "idx_cols": idx_total_cols,
        "st_cols": st_total_cols,
        "idx": idx_arrs,
        "st": st_arrs,
        "dinvsq": dinvsq,
    }


def _build(plan):
    idx_cols = plan["idx_cols"]
    st_cols = plan["st_cols"]
    calls = plan["calls"]

    nc = bacc.Bacc(
        "TRN2", target_bir_lowering=False, debug=False, num_devices=N_CORES
    )
    xtl = nc.dram_tensor("xtl", [TILES * 128, F0], BF16, kind="ExternalInput")
    w1 = nc.dram_tensor("w1", [F0, FM], BF16, kind="ExternalInput")
    w2 = nc.dram_tensor("w2", [FM, FM], BF16, kind="ExternalInput")
    b1c = nc.dram_tensor("b1c", [128, 2], F32, kind="ExternalInput")
    b2bc = nc.dram_tensor("b2bc", [128, FM], F32, kind="ExternalInput")
    dsq = nc.dram_tensor("dsq", [128, TILES], F32, kind="ExternalInput")
    idn = nc.dram_tensor("idn", [128, 128], BF16, kind="ExternalInput")
    idxs = nc.dram_tensor("idxs", [32, idx_cols], I16, kind="ExternalInput")
    stl = nc.dram_tensor("stl", [128, st_cols * 128], BF16, kind="ExternalInput")
    out = nc.dram_tensor("out", [NPC, FM], F32, kind="ExternalOutput")

    K0 = F0 // 128  # 6
    ACUT = 25 * 128  # rows 0..3200 cover shard half A (0..3125)

    with tile.TileContext(nc) as tc:
        with (
            tc.tile_pool(name="const", bufs=1) as cpool,
            tc.tile_pool(name="sbuf", bufs=3) as sbuf,
            tc.tile_pool(name="xbuf", bufs=3) as xbuf,
            tc.tile_pool(name="gbuf", bufs=2) as gbuf,
            tc.tile_pool(name="stbuf", bufs=2) as stbuf,
            tc.tile_pool(name="psum", bufs=2, space="PSUM") as psum,
            tc.tile_pool(name="dram", bufs=1, space="DRAM") as dram,
        ):
            # ---- persistent tiles ----
            w1t = cpool.tile([128, K0, FM], BF16)
            nc.sync.dma_start(
                out=w1t[:], in_=w1[:].rearrange("(k p) f -> p k f", p=128)
            )
            w2t = cpool.tile([128, 2, FM], BF16)
            nc.sync.dma_start(
                out=w2t[:], in_=w2[:].rearrange("(k p) f -> p k f", p=128)
            )
            b1t = cpool.tile([128, 2], F32)
            nc.sync.dma_start(out=b1t[:], in_=b1c[:])
            b2t = cpool.tile([128, FM], F32)
            nc.sync.dma_start(out=b2t[:], in_=b2bc[:])
            dvt = cpool.tile([128, TILES], F32)
            nc.sync.dma_start(out=dvt[:], in_=dsq[:])
            idt = cpool.tile([128, 128], BF16)
            nc.sync.dma_start(out=idt[:], in_=idn[:])
            idx_t = cpool.tile([32, idx_cols], I16)
            nc.sync.dma_start(out=idx_t[:], in_=idxs[:])
            h1own = cpool.tile([128, TILES, FM], BF16)
            nc.vector.memset(h1own[:], 0.0)
            h2own = cpool.tile([128, TILES, FM], BF16)
            nc.vector.memset(h2own[:], 0.0)

            h1sA = dram.tile([HALF, FM], BF16)
            h1sB = dram.tile([NPC - HALF, FM], BF16)
            h1fA = dram.tile([NT, FM], BF16, addr_space="Shared")
            h1fB = dram.tile([N - NT, FM], BF16, addr_space="Shared")
            h2sA = dram.tile([HALF, FM], BF16)
            h2sB = dram.tile([NPC - HALF, FM], BF16)
            h2fA = dram.tile([NT, FM], BF16, addr_space="Shared")
            h2fB = dram.tile([N - NT, FM], BF16, addr_space="Shared")

            def tw_of(t):
                return min(128, NPC - t * 128)

            def write_shard(hbuf, t, shardA, shardB):
                # rows t*128 .. t*128+tw of the shard, split at HALF
                tw = tw_of(t)
                r0, r1 = t * 128, t * 128 + tw
                if r1 <= HALF:
                    nc.sync.dma_start(
                        out=shardA[r0:r1, :], in_=hbuf[:tw, t, :]
                    )
                elif r0 >= HALF:
                    nc.sync.dma_start(
                        out=shardB[r0 - HALF : r1 - HALF, :], in_=hbuf[:tw, t, :]
                    )
                else:
                    m = HALF - r0
                    nc.sync.dma_start(
                        out=shardA[r0:HALF, :], in_=hbuf[:m, t, :]
                    )
                    nc.sync.dma_start(
                        out=shardB[0 : r1 - HALF, :], in_=hbuf[m:tw, t, :]
                    )

            # ---- P1: h1 = x @ W1 (unscaled), kept on-chip + sharded out ----
            def p1_tile(t):
                tw = tw_of(t)
                xt = xbuf.tile([128, K0, 128], BF16, tag="xt")
                nc.sync.dma_start(
                    out=xt[:],
                    in_=xtl[t * 128 : (t + 1) * 128, :].rearrange(
                        "p (k j) -> p k j", j=128
                    ),
                )
                ps = psum.tile([128, FM], F32, tag="mmps", space="PSUM")
                for k in range(K0):
                    nc.tensor.matmul(
                        out=ps[:tw, :],
                        lhsT=xt[:, k, :tw],
                        rhs=w1t[:, k, :],
                        start=(k == 0),
                        stop=(k == K0 - 1),
                    )
                nc.scalar.activation(
                    out=h1own[:tw, t, :],
                    in_=ps[:tw, :],
                    func=mybir.ActivationFunctionType.Copy,
                )
                write_shard(h1own, t, h1sA, h1sB)

            for t in range(25):
                p1_tile(t)
            nc.gpsimd.collective_compute(
                "AllGather",
                mybir.AluOpType.bypass,
                replica_groups=[list(range(N_CORES))],
                ins=[h1sA.opt()],
                outs=[h1fA.opt()],
            )
            for t in range(25, TILES):
                p1_tile(t)
            nc.gpsimd.collective_compute(
                "AllGather",
                mybir.AluOpType.bypass,
                replica_groups=[list(range(N_CORES))],
                ins=[h1sB.opt()],
                outs=[h1fB.opt()],
            )

            # ---- gather + st helpers ----
            def gather_call(call, table, tag):
                g = gbuf.tile([128, call["nch"], FM], BF16, tag=tag)
                nc.gpsimd.dma_gather(
                    g[:],
                    table[:, :],
                    idx_t[:, call["idx_off"] : call["idx_off"] + call["rows"] // 16],
                    call["rows"],
                    call["rows"],
                    FM,
                    single_packet=False,
                )
                return g

            def load_st(sg):
                cA = calls[2 * sg]
                cB = calls[2 * sg + 1]
                nb = cA["nblocks"] + cB["nblocks"]
                st = stbuf.tile([128, nb * 128], BF16, tag="st")
                nc.sync.dma_start(
                    out=st[:],
                    in_=stl[:, cA["st_off"] * 128 : (cA["st_off"] + nb) * 128],
                )
                return st, cA["st_off"]

            # ---- P2: layer-1 message passing + layer-2 dense ----
            def p2_sg(sg, gA, gB, st, st_base):
                cA = calls[2 * sg]
                cB = calls[2 * sg + 1]
                for ti in range(len(cA["tiles"])):
                    t, ksA, coffA = cA["tiles"][ti]
                    _, ksB, coffB = cB["tiles"][ti]
                    tw = tw_of(t)
                    tmp = sbuf.tile([128, FM], BF16, tag="tmp")
                    nc.scalar.activation(
                        out=tmp[:],
                        in_=h1own[:, t, :],
                        func=mybir.ActivationFunctionType.Copy,
                        scale=dvt[:, t : t + 1],
                    )
                    ps0 = psum.tile([128, 128], F32, tag="psT0", space="PSUM")
                    ps1 = psum.tile([128, 128], F32, tag="psT1", space="PSUM")
                    # self-loop: transpose dinv2*h1own into [feat, dst] psum
                    nc.tensor.matmul(
                        out=ps0[:, :tw],
                        lhsT=tmp[:, 0:128],
                        rhs=idt[:, :tw],
                        start=True,
                        stop=False,
                    )
                    nc.tensor.matmul(
                        out=ps1[:, :tw],
                        lhsT=tmp[:, 128:256],
                        rhs=idt[:, :tw],
                        start=True,
                        stop=False,
                    )
                    seq = [(gA, ksA, coffA), (gB, ksB, coffB)]
                    ntot = len(ksA) + len(ksB)
                    i = 0
                    for g, ks, coff in seq:
                        for j, k in enumerate(ks):
                            off = (coff - st_base + j) * 128
                            last = i == ntot - 1
                            nc.tensor.matmul(
                                out=ps0[:, :tw],
                                lhsT=g[:, k, 0:128],
                                rhs=st[:, off : off + tw],
                                start=False,
                                stop=last,
                            )
                            nc.tensor.matmul(
                                out=ps1[:, :tw],
                                lhsT=g[:, k, 128:256],
                                rhs=st[:, off : off + tw],
                                start=False,
                                stop=last,
                            )
                            i += 1
                    x1a = sbuf.tile([128, 128], BF16, tag="x1a")
                    x1b = sbuf.tile([128, 128], BF16, tag="x1b")
                    nc.scalar.activation(
                        out=x1a[:, :tw],
                        in_=ps0[:, :tw],
                        func=mybir.ActivationFunctionType.Relu,
                        bias=b1t[:, 0:1],
                    )
                    nc.scalar.activation(
                        out=x1b[:, :tw],
                        in_=ps1[:, :tw],
                        func=mybir.ActivationFunctionType.Relu,
                        bias=b1t[:, 1:2],
                    )
                    ps2 = psum.tile([128, FM], F32, tag="mmps", space="PSUM")
                    nc.tensor.matmul(
                        out=ps2[:tw, :],
                        lhsT=x1a[:, :tw],
                        rhs=w2t[:, 0, :],
                        start=True,
                        stop=False,
                    )
                    nc.tensor.matmul(
                        out=ps2[:tw, :],
                        lhsT=x1b[:, :tw],
                        rhs=w2t[:, 1, :],
                        start=False,
                        stop=True,
                    )
                    nc.scalar.activation(
                        out=h2own[:tw, t, :],
                        in_=ps2[:tw, :],
                        func=mybir.ActivationFunctionType.Copy,
                    )
                    write_shard(h2own, t, h2sA, h2sB)

            def run_phase(proc, tblA, tblB):
                # pipelined: [gA0], then per sg: [st_i, gA_{i+1}, gB_i, proc_i]
                gAs = {0: gather_call(calls[0], tblA, "gA")}
                for sg in range(NSG):
                    st, st_base = load_st(sg)
                    if sg + 1 < NSG:
                        gAs[sg + 1] = gather_call(
                            calls[2 * (sg + 1)], tblA, "gA"
                        )
                    gB = gather_call(calls[2 * sg + 1], tblB, "gB")
                    proc(sg, gAs.pop(sg), gB, st, st_base)

            # sg containing tile 24 (end of shard half A)
            sg24 = 24 // SGT

            def p2_proc(sg, gA, gB, st, st_base):
                p2_sg(sg, gA, gB, st, st_base)
                if sg == sg24:
                    nc.gpsimd.collective_compute(
                        "AllGather",
                        mybir.AluOpType.bypass,
                        replica_groups=[list(range(N_CORES))],
                        ins=[h2sA.opt()],
                        outs=[h2fA.opt()],
                    )

            run_phase(p2_proc, h1fA, h1fB)
            nc.gpsimd.collective_compute(
                "AllGather",
                mybir.AluOpType.bypass,
                replica_groups=[list(range(N_CORES))],
                ins=[h2sB.opt()],
                outs=[h2fB.opt()],
            )

            # ---- P3: layer-2 message passing + self-loop + bias ----
            def p3_sg(sg, gA, gB, st, st_base):
                cA = calls[2 * sg]
                cB = calls[2 * sg + 1]
                for ti in range(len(cA["tiles"])):
                    t, ksA, coffA = cA["tiles"][ti]
                    _, ksB, coffB = cB["tiles"][ti]
                    tw = tw_of(t)
                    ps = psum.tile([128, FM], F32, tag="mmps", space="PSUM")
                    seq = [(gA, ksA, coffA), (gB, ksB, coffB)]
                    ntot = len(ksA) + len(ksB)
                    i = 0
                    for g, ks, coff in seq:
                        for j, k in enumerate(ks):
                            off = (coff - st_base + j) * 128
                            nc.tensor.matmul(
                                out=ps[:tw, :],
                                lhsT=st[:, off : off + tw],
                                rhs=g[:, k, :],
                                start=(i == 0),
                                stop=(i == ntot - 1),
                            )
                            i += 1
                    tmp = sbuf.tile([128, FM], F32, tag="tmp3")
                    nc.scalar.activation(
                        out=tmp[:tw, :],
                        in_=h2own[:tw, t, :],
                        func=mybir.ActivationFunctionType.Copy,
                        scale=dvt[:tw, t : t + 1],
                    )
                    ot = sbuf.tile([128, FM], F32, tag="ot")
                    nc.vector.tensor_add(
                        out=ot[:tw, :], in0=ps[:tw, :], in1=tmp[:tw, :]
                    )
                    nc.vector.tensor_add(
                        out=ot[:tw, :], in0=ot[:tw, :], in1=b2t[:tw, :]
                    )
                    nc.sync.dma_start(
                        out=out[t * 128 : t * 128 + tw, :], in_=ot[:tw, :]
                    )

            run_phase(p3_sg, h2fA, h2fB)
    nc.compile()
    return nc


def _prep(plan, x, W1, b1, W2, b2):
    x = np.asarray(x, np.float32)
    W1 = np.asarray(W1, np.float32).astype(ml_dtypes.bfloat16)
    W2 = np.asarray(W2, np.float32).astype(ml_dtypes.bfloat16)
    b1 = np.asarray(b1, np.float32)
    b2 = np.asarray(b2, np.float32)
    b1c = np.ascontiguousarray(b1.reshape(2, 128).T)
    b2bc = np.ascontiguousarray(np.broadcast_to(b2[None, :], (128, FM)))
    idn = np.eye(128, dtype=np.float32).astype(ml_dtypes.bfloat16)
    in_maps = []
    for c in range(N_CORES):
        xs = x[c * NPC : (c + 1) * NPC]
        # xtl[t*128 + p, k*128 + j] = xs[t*128 + j, k*128 + p]
        xp = np.zeros((TILES * 128, F0), np.float32)
        xsp = np.zeros((TILES * 128, F0), np.float32)
        xsp[:NPC] = xs
        xt4 = xsp.reshape(TILES, 128, K0F := F0 // 128, 128)  # [t, j, k, p]
        xp = xt4.transpose(0, 3, 2, 1).reshape(TILES * 128, F0)
        in_maps.append(
            {
                "xtl": np.ascontiguousarray(xp).astype(ml_dtypes.bfloat16),
                "w1": W1,
                "w2": W2,
                "b1c": b1c,
                "b2bc": b2bc,
                "dsq": plan["dinvsq"][c],
                "idn": idn,
                "idxs": plan["idx"][c],
                "stl": plan["st"][c],
            }
        )
    return in_maps


def kernel(x, edge_index, W1, b1, W2, b2):
    key = hashlib.sha256(np.asarray(edge_index).tobytes()).hexdigest()
    if key not in _cache:
        plan = _make_plan(edge_index)
        nc = _build(plan)
        _cache[key] = (plan, nc)
    plan, nc = _cache[key]
    in_maps = _prep(plan, x, W1, b1, W2, b2)

    last_err = None
    for _ in range(3):
        try:
            res = run_bass_kernel_spmd(
                nc, in_maps, core_ids=list(range(N_CORES))
            )
            break
        except Exception as e:  # transient NRT failures
            last_err = e
    else:
        raise last_err
    return np.concatenate([res.results[c]["out"] for c in range(N_CORES)], axis=0)
